# revision 5
# baseline (speedup 1.0000x reference)
"""GQA attention (dense transformer block) for 8 Trainium2 NeuronCores.

Sharding: tensor-parallel over heads. Core c owns KV head c (of 8) and the
4 query heads grouped on it; wq/wk/wv are sharded on their output dim,
wo on its input dim. Each core computes a full [B,S,DIM] partial of the
output projection; the host sums the 8 partials (the unshard step for
input-dim sharding of wo).

All matmuls run in float32r (FP22) at 1 cycle/row.
"""

import os
import sys
import numpy as np

for _p in ("/opt/trn_rl_repo", "/root/.axon_site/_ro/trn_rl_repo"):
    if os.path.isdir(_p) and _p not in sys.path:
        sys.path.insert(0, _p)

import concourse.bass as bass  # noqa: E402
import concourse.mybir as mybir  # noqa: E402
import concourse.tile as tile  # noqa: E402
from concourse import bacc  # noqa: E402
from concourse.bass_utils import run_bass_kernel_spmd  # noqa: E402
from concourse.masks import make_identity  # noqa: E402

F32 = mybir.dt.float32
F32R = mybir.dt.float32r

# Problem constants (full problem; the builder below is parameterized so a
# scaled-down config can run under CoreSim).
B, S, DIM = 2, 2048, 4096
NH, NKV, HD = 32, 8, 128
NCORES = 8
NQH = NH // NKV  # q heads per core = 4


class Cfg:
    def __init__(self, B=B, S=S, DIM=DIM, NQH=NQH, causal=True):
        self.B, self.S, self.DIM, self.NQH = B, S, DIM, NQH
        self.causal = causal
        self.TB = 512                 # token block (matmul moving width)
        self.NB = S // self.TB        # token blocks per batch row
        self.NDT = DIM // 128         # contraction tiles for projections
        self.NKT = S // 128           # key tiles per batch row
        self.SCALE = 1.0 / float(np.sqrt(HD))


def build_gqa_nc(cfg: Cfg):
    """One SPMD program; per-core differences come in through the inputs."""
    nc = bacc.Bacc("TRN2", target_bir_lowering=False)
    TB, NB, NDT, NKT = cfg.TB, cfg.NB, cfg.NDT, cfg.NKT
    Bc, Sc, DIMc, NQHc = cfg.B, cfg.S, cfg.DIM, cfg.NQH

    xT = nc.declare_dram_parameter("xT", [DIMc, Bc * Sc], F32R, isOutput=False)
    wq = nc.declare_dram_parameter("wq", [DIMc, NQHc * HD], F32R, isOutput=False)
    wk = nc.declare_dram_parameter("wk", [DIMc, HD], F32R, isOutput=False)
    wv = nc.declare_dram_parameter("wv", [DIMc, HD], F32R, isOutput=False)
    wo = nc.declare_dram_parameter("wo", [NQHc * HD, DIMc], F32R, isOutput=False)
    cosT = nc.declare_dram_parameter("cosT", [HD, Sc], F32, isOutput=False)
    sinTs = nc.declare_dram_parameter("sinTs", [HD, Sc], F32, isOutput=False)
    if cfg.causal:
        # 4 universal diagonal-block tiles: maskd[j][k, q]
        maskd = nc.declare_dram_parameter("maskd", [4, 128, TB], F32, isOutput=False)
    else:
        maskf = nc.declare_dram_parameter(
            "maskf", [NB, NKT, 128, TB], F32, isOutput=False
        )

    outp = nc.declare_dram_parameter("outp", [Bc, Sc, DIMc], F32, isOutput=True)
    ko = nc.declare_dram_parameter("ko", [Bc, Sc, HD], F32, isOutput=True)
    vo = nc.declare_dram_parameter("vo", [Bc, Sc, HD], F32, isOutput=True)

    wq_r = wq.rearrange("(t p) h -> p t h", p=128)  # [128, NDT, NQH*HD]
    wk_r = wk.rearrange("(t p) h -> p t h", p=128)
    wv_r = wv.rearrange("(t p) h -> p t h", p=128)
    wo_r = wo.rearrange("(h p) d -> p h d", p=128)  # [128, NQH, DIM]

    with tile.TileContext(nc) as tc:
        import contextlib

        with contextlib.ExitStack() as ctx:
            consts = ctx.enter_context(tc.tile_pool(name="consts", bufs=1))
            ktv = ctx.enter_context(tc.tile_pool(name="ktv", bufs=1))
            attnp = ctx.enter_context(tc.tile_pool(name="attnp", bufs=1))
            dramp = ctx.enter_context(tc.tile_pool(name="dramp", bufs=1, space="DRAM"))

            # ---- constants ----
            cos_sb = consts.tile([128, Sc], F32, tag="cos")
            sin_sb = consts.tile([128, Sc], F32, tag="sin")
            nc.sync.dma_start(out=cos_sb, in_=cosT[:, :])
            nc.sync.dma_start(out=sin_sb, in_=sinTs[:, :])
            ident_f = consts.tile([128, 128], F32, tag="identf")
            make_identity(nc, ident_f)
            ident = consts.tile([128, 128], F32R, tag="ident")
            nc.vector.tensor_copy(ident, ident_f)
            ones_f = consts.tile([128, 128], F32, tag="onesf")
            nc.vector.memset(ones_f, 1.0)
            ones = consts.tile([128, 128], F32R, tag="ones")
            nc.vector.tensor_copy(ones, ones_f)

            for b in range(Bc):
                KTr = ktv.tile([128, Sc], F32R, tag="ktr")       # rotated K^T
                Vb = ktv.tile([128, NKT, 128], F32R, tag="v")     # V [tok,hd] tiles
                attnT = attnp.tile([128, NQHc, Sc], F32R, tag="attnT")
                qts = dramp.tile([NQHc, 128, Sc], F32R, tag="qts")  # QT spill

                # ================= phase 1: projections + RoPE =================
                with contextlib.ExitStack() as p1:
                    wp = p1.enter_context(tc.tile_pool(name="wp", bufs=1))
                    xs = p1.enter_context(tc.tile_pool(name="xs", bufs=3))
                    stg = p1.enter_context(tc.tile_pool(name="stg", bufs=2))
                    psp = p1.enter_context(
                        tc.tile_pool(name="psp", bufs=1, space="PSUM")
                    )
                    pst = p1.enter_context(
                        tc.tile_pool(name="pst", bufs=2, space="PSUM")
                    )

                    wq_sb = wp.tile([128, NDT, NQHc * HD], F32R, tag="wq")
                    wk_sb = wp.tile([128, NDT, HD], F32R, tag="wk")
                    wv_sb = wp.tile([128, NDT, HD], F32R, tag="wv")
                    nc.sync.dma_start(out=wq_sb, in_=wq_r)
                    nc.sync.dma_start(out=wk_sb, in_=wk_r)
                    nc.sync.dma_start(out=wv_sb, in_=wv_r)

                    for tb in range(NB):
                        tsl = slice(tb * TB, (tb + 1) * TB)
                        psq = [psp.tile([128, TB], F32, tag=f"q{i}", name=f"psq{i}") for i in range(NQHc)]
                        psk = psp.tile([128, TB], F32, tag="k")
                        psv = psp.tile([128, TB], F32, tag="vv")
                        for dt in range(NDT):
                            xt = xs.tile([128, TB], F32R, tag="xt")
                            nc.sync.dma_start(
                                out=xt,
                                in_=xT[dt * 128:(dt + 1) * 128,
                                       b * Sc + tb * TB: b * Sc + (tb + 1) * TB],
                            )
                            st_ = dt == 0
                            sp_ = dt == NDT - 1
                            for i in range(NQHc):
                                nc.tensor.matmul(
                                    psq[i], wq_sb[:, dt, i * HD:(i + 1) * HD], xt,
                                    start=st_, stop=sp_,
                                )
                            nc.tensor.matmul(psk, wk_sb[:, dt, :], xt, start=st_, stop=sp_)
                            nc.tensor.matmul(psv, wv_sb[:, dt, :], xt, start=st_, stop=sp_)

                        ct = cos_sb[:, tsl]
                        st = sin_sb[:, tsl]

                        def rope(ps_in, out_ap):
                            # out = ps_in * cos + rot_half(ps_in) * sin
                            # sinTs rows 0:64 hold -sin, rows 64:128 hold +sin.
                            tmp = stg.tile([128, TB], F32, tag="ropetmp")
                            nc.vector.tensor_mul(tmp[0:64, :], ps_in[64:128, :], st[0:64, :])
                            nc.vector.tensor_mul(tmp[64:128, :], ps_in[0:64, :], st[64:128, :])
                            tmp2 = stg.tile([128, TB], F32, tag="ropetmp2")
                            nc.vector.tensor_mul(tmp2, ps_in, ct)
                            nc.vector.tensor_add(out_ap, tmp2, tmp)

                        for i in range(NQHc):
                            qt_s = stg.tile([128, TB], F32R, tag="qt_s")
                            rope(psq[i], qt_s)
                            nc.sync.dma_start(out=qts[i, :, tsl], in_=qt_s)
                        rope(psk, KTr[:, tsl])

                        vt_s = stg.tile([128, TB], F32R, tag="vt_s")
                        nc.vector.tensor_copy(vt_s, psv)

                        for j in range(TB // 128):
                            tt = tb * (TB // 128) + j
                            tp = pst.tile([128, 128], F32R, tag="tr")
                            nc.tensor.transpose(tp, vt_s[:, j * 128:(j + 1) * 128], ident)
                            vtile = Vb[:, tt, :]
                            nc.vector.tensor_copy(vtile, tp)
                            nc.sync.dma_start(
                                out=vo[b, tt * 128:(tt + 1) * 128, :],
                                in_=vtile.bitcast(F32),
                            )
                            ktp = pst.tile([128, 128], F32R, tag="tr", name="ktp")
                            nc.tensor.transpose(
                                ktp, KTr[:, tt * 128:(tt + 1) * 128], ident
                            )
                            ko_s = stg.tile([128, 128], F32, tag="ko_s")
                            nc.scalar.copy(ko_s, ktp.bitcast(F32))
                            nc.sync.dma_start(
                                out=ko[b, tt * 128:(tt + 1) * 128, :], in_=ko_s
                            )

                # ================= phase 2: attention =================
                with contextlib.ExitStack() as p2:
                    qtp = p2.enter_context(tc.tile_pool(name="qtp", bufs=1))
                    exp_ = p2.enter_context(tc.tile_pool(name="exp", bufs=3))
                    st2 = p2.enter_context(tc.tile_pool(name="st2", bufs=2))
                    mkp = p2.enter_context(tc.tile_pool(name="mkp", bufs=1))
                    pssc = p2.enter_context(
                        tc.tile_pool(name="pssc", bufs=3, space="PSUM")
                    )
                    pspv = p2.enter_context(
                        tc.tile_pool(name="pspv", bufs=2, space="PSUM")
                    )
                    psdn = p2.enter_context(
                        tc.tile_pool(name="psdn", bufs=2, space="PSUM")
                    )
                    if cfg.causal:
                        mk_sb = mkp.tile([128, 4, TB], F32, tag="mk")
                        nc.sync.dma_start(
                            out=mk_sb, in_=maskd.rearrange("j p q -> p j q")
                        )
                    else:
                        mstr = p2.enter_context(tc.tile_pool(name="mstr", bufs=3))

                    for h in range(NQHc):
                        qth = qtp.tile([128, Sc], F32R, tag="qth")
                        nc.sync.dma_start(out=qth, in_=qts[h, :, :])
                        for qb in range(NB):
                            n_kt = 4 * (qb + 1) if cfg.causal else NKT
                            qsl = slice(qb * TB, (qb + 1) * TB)
                            pv = pspv.tile([128, TB], F32, tag="pv")
                            dn = psdn.tile([128, TB], F32, tag="dn")
                            for kt in range(n_kt):
                                sc = pssc.tile([128, TB], F32, tag="sc")
                                nc.tensor.matmul(
                                    sc, KTr[:, kt * 128:(kt + 1) * 128], qth[:, qsl],
                                    start=True, stop=True,
                                )
                                if cfg.causal:
                                    j = kt - (n_kt - 4)
                                    if j >= 0:
                                        nc.vector.tensor_add(sc, sc, mk_sb[:, j, :])
                                else:
                                    mt = mstr.tile([128, TB], F32, tag="mt")
                                    nc.sync.dma_start(out=mt, in_=maskf[qb, kt, :, :])
                                    nc.vector.tensor_add(sc, sc, mt)
                                ex = exp_.tile([128, TB], F32R, tag="ex")
                                nc.scalar.activation(
                                    ex, sc, mybir.ActivationFunctionType.Exp,
                                    bias=0.0, scale=cfg.SCALE,
                                )
                                st_ = kt == 0
                                sp_ = kt == n_kt - 1
                                nc.tensor.matmul(dn, ones, ex, start=st_, stop=sp_)
                                nc.tensor.matmul(pv, Vb[:, kt, :], ex, start=st_, stop=sp_)
                            rec = st2.tile([128, TB], F32, tag="rec")
                            nc.vector.reciprocal(rec, dn)
                            nc.vector.tensor_mul(attnT[:, h, qsl], pv, rec)

                # ================= phase 3: output projection =================
                with contextlib.ExitStack() as p3:
                    wop = p3.enter_context(tc.tile_pool(name="wop", bufs=2))
                    ost = p3.enter_context(tc.tile_pool(name="ost", bufs=3))
                    pso = p3.enter_context(
                        tc.tile_pool(name="pso", bufs=4, space="PSUM")
                    )
                    for db in range(DIMc // TB):
                        dsl = slice(db * TB, (db + 1) * TB)
                        wos = wop.tile([128, NQHc, TB], F32R, tag="wos")
                        nc.sync.dma_start(out=wos, in_=wo_r[:, :, dsl])
                        for tt in range(NKT):
                            po = pso.tile([128, TB], F32, tag="po")
                            for h in range(NQHc):
                                nc.tensor.matmul(
                                    po, attnT[:, h, tt * 128:(tt + 1) * 128],
                                    wos[:, h, :],
                                    start=(h == 0), stop=(h == NQHc - 1),
                                )
                            os_ = ost.tile([128, TB], F32)
                            nc.scalar.copy(os_, po)
                            nc.sync.dma_start(
                                out=outp[b, tt * 128:(tt + 1) * 128, dsl], in_=os_
                            )

    nc.compile()
    return nc


# --------------------------------------------------------------------------
# host side
# --------------------------------------------------------------------------

_CACHE = {}


def _get_nc(causal: bool):
    if causal not in _CACHE:
        _CACHE[causal] = build_gqa_nc(Cfg(causal=causal))
    return _CACHE[causal]


def _host_prep(x, cos, sin, mask, wq, wk, wv, wo):
    x = np.asarray(x, dtype=np.float32)
    cos = np.asarray(cos, dtype=np.float32)
    sin = np.asarray(sin, dtype=np.float32)
    mask = np.asarray(mask, dtype=np.float32)
    wq = np.asarray(wq, dtype=np.float32)
    wk = np.asarray(wk, dtype=np.float32)
    wv = np.asarray(wv, dtype=np.float32)
    wo = np.asarray(wo, dtype=np.float32)

    xT = np.ascontiguousarray(x.reshape(B * S, DIM).T)  # [DIM, B*S]
    cosT = np.ascontiguousarray(cos.T)  # [HD, S]
    sinT = cos.T * 0  # placeholder alloc avoided; build directly below
    sinT = np.ascontiguousarray(sin.T)
    sinTs = np.concatenate([-sinT[: HD // 2], sinT[HD // 2:]], axis=0)
    sinTs = np.ascontiguousarray(sinTs)

    # causal structure check
    tril = np.tril(np.ones((S, S), dtype=bool))
    causal = bool(np.all(mask[tril] == 0.0) and np.all(mask[~tril] <= -1e8))

    if causal:
        TB = 512
        kk = np.arange(128)[:, None]
        qq = np.arange(TB)[None, :]
        maskd = np.stack(
            [np.where(qq >= j * 128 + kk, 0.0, -1e9).astype(np.float32)
             for j in range(4)]
        )
        mask_inputs = {"maskd": maskd}
    else:
        TB = 512
        mT = np.ascontiguousarray(mask.T)  # [k, q]
        maskf = np.ascontiguousarray(
            mT.reshape(S // 128, 128, S // TB, TB).transpose(2, 0, 1, 3)
        )
        mask_inputs = {"maskf": maskf}

    in_maps = []
    for c in range(NCORES):
        qsl = slice(c * NQH * HD, (c + 1) * NQH * HD)
        ksl = slice(c * HD, (c + 1) * HD)
        in_maps.append(
            {
                "xT": xT,
                "wq": np.ascontiguousarray(wq[:, qsl]),
                "wk": np.ascontiguousarray(wk[:, ksl]),
                "wv": np.ascontiguousarray(wv[:, ksl]),
                "wo": np.ascontiguousarray(wo[qsl, :]),
                "cosT": cosT,
                "sinTs": sinTs,
                **mask_inputs,
            }
        )
    return in_maps, causal


def run_device(x, cos, sin, mask, wq, wk, wv, wo, trace=False):
    in_maps, causal = _host_prep(x, cos, sin, mask, wq, wk, wv, wo)
    nc = _get_nc(causal)
    res = run_bass_kernel_spmd(nc, in_maps, list(range(NCORES)), trace=trace)
    return res


def kernel(x, cos, sin, mask, wq, wk, wv, wo):
    res = run_device(x, cos, sin, mask, wq, wk, wv, wo, trace=False)
    outs = res.results
    out = np.zeros((B, S, DIM), dtype=np.float64)
    for c in range(NCORES):
        out += outs[c]["outp"].astype(np.float64)
    out = out.astype(np.float32)
    k = np.stack([outs[c]["ko"] for c in range(NCORES)], axis=1)  # [B,NKV,S,HD]
    v = np.stack([outs[c]["vo"] for c in range(NCORES)], axis=1)
    return out, k, v


# revision 6
# speedup vs baseline: 1.0546x; 1.0546x over previous
"""GQA attention (dense transformer block) for 8 Trainium2 NeuronCores.

Sharding: tensor-parallel over heads. Core c owns KV head c (of 8) and the
4 query heads grouped on it; wq/wk/wv are sharded on their output dim,
wo on its input dim. Each core computes a full [B,S,DIM] partial of the
output projection; the host sums the 8 partials (the unshard step for
input-dim sharding of wo).

Matmul operand dtype is selectable: float32r (FP22, ~2 cyc/row) or
bfloat16 (1 cyc/row, FWL weight loads). PSUM accumulation is fp32 either
way.
"""

import os
import sys
import numpy as np

for _p in ("/opt/trn_rl_repo", "/root/.axon_site/_ro/trn_rl_repo"):
    if os.path.isdir(_p) and _p not in sys.path:
        sys.path.insert(0, _p)

import concourse.bass as bass  # noqa: E402
import concourse.mybir as mybir  # noqa: E402
import concourse.tile as tile  # noqa: E402
from concourse import bacc  # noqa: E402
from concourse.bass_utils import run_bass_kernel_spmd  # noqa: E402
from concourse.masks import make_identity  # noqa: E402

F32 = mybir.dt.float32
F32R = mybir.dt.float32r
BF16 = mybir.dt.bfloat16

# Problem constants (full problem; the builder below is parameterized so a
# scaled-down config can run under CoreSim).
B, S, DIM = 2, 2048, 4096
NH, NKV, HD = 32, 8, 128
NCORES = 8
NQH = NH // NKV  # q heads per core = 4


class Cfg:
    def __init__(self, B=B, S=S, DIM=DIM, NQH=NQH, causal=True, dtype="bf16"):
        self.B, self.S, self.DIM, self.NQH = B, S, DIM, NQH
        self.causal = causal
        self.dtype = dtype
        self.MDT = BF16 if dtype == "bf16" else F32R  # matmul operand dtype
        self.TB = 512                 # token block (matmul moving width)
        self.NB = S // self.TB        # token blocks per batch row
        self.NDT = DIM // 128         # contraction tiles for projections
        self.NKT = S // 128           # key tiles per batch row
        self.SCALE = 1.0 / float(np.sqrt(HD))


def build_gqa_nc(cfg: Cfg):
    """One SPMD program; per-core differences come in through the inputs."""
    nc = bacc.Bacc("TRN2", target_bir_lowering=False)
    TB, NB, NDT, NKT = cfg.TB, cfg.NB, cfg.NDT, cfg.NKT
    Bc, Sc, DIMc, NQHc = cfg.B, cfg.S, cfg.DIM, cfg.NQH
    MDT = cfg.MDT

    xT = nc.declare_dram_parameter("xT", [DIMc, Bc * Sc], MDT, isOutput=False)
    wq = nc.declare_dram_parameter("wq", [DIMc, NQHc * HD], MDT, isOutput=False)
    wk = nc.declare_dram_parameter("wk", [DIMc, HD], MDT, isOutput=False)
    wv = nc.declare_dram_parameter("wv", [DIMc, HD], MDT, isOutput=False)
    wo = nc.declare_dram_parameter("wo", [NQHc * HD, DIMc], MDT, isOutput=False)
    cosT = nc.declare_dram_parameter("cosT", [HD, Sc], F32, isOutput=False)
    sinTs = nc.declare_dram_parameter("sinTs", [HD, Sc], F32, isOutput=False)
    if cfg.causal:
        # 4 universal diagonal-block tiles: maskd[j][k, q]
        maskd = nc.declare_dram_parameter("maskd", [4, 128, TB], F32, isOutput=False)
    else:
        maskf = nc.declare_dram_parameter(
            "maskf", [NB, NKT, 128, TB], F32, isOutput=False
        )

    outp = nc.declare_dram_parameter("outp", [Bc, Sc, DIMc], F32, isOutput=True)
    ko = nc.declare_dram_parameter("ko", [Bc, Sc, HD], F32, isOutput=True)
    vo = nc.declare_dram_parameter("vo", [Bc, Sc, HD], F32, isOutput=True)

    wq_r = wq.rearrange("(t p) h -> p t h", p=128)  # [128, NDT, NQH*HD]
    wk_r = wk.rearrange("(t p) h -> p t h", p=128)
    wv_r = wv.rearrange("(t p) h -> p t h", p=128)
    wo_r = wo.rearrange("(h p) d -> p h d", p=128)  # [128, NQH, DIM]

    with tile.TileContext(nc) as tc:
        import contextlib

        with contextlib.ExitStack() as ctx:
            consts = ctx.enter_context(tc.tile_pool(name="consts", bufs=1))
            wpool = ctx.enter_context(tc.tile_pool(name="wpool", bufs=1))
            ktv = ctx.enter_context(tc.tile_pool(name="ktv", bufs=1))
            attnp = ctx.enter_context(tc.tile_pool(name="attnp", bufs=1))
            dramp = ctx.enter_context(tc.tile_pool(name="dramp", bufs=1, space="DRAM"))

            # ---- constants ----
            cos_sb = consts.tile([128, Sc], F32, tag="cos")
            sin_sb = consts.tile([128, Sc], F32, tag="sin")
            nc.sync.dma_start(out=cos_sb, in_=cosT[:, :])
            nc.sync.dma_start(out=sin_sb, in_=sinTs[:, :])
            ident_f = consts.tile([128, 128], F32, tag="identf")
            make_identity(nc, ident_f)
            ident = consts.tile([128, 128], MDT, tag="ident")
            nc.vector.tensor_copy(ident, ident_f)
            ones_f = consts.tile([128, 128], F32, tag="onesf")
            nc.vector.memset(ones_f, 1.0)
            ones = consts.tile([128, 128], MDT, tag="ones")
            nc.vector.tensor_copy(ones, ones_f)

            # ---- weights: loaded once, resident for the whole kernel ----
            # chunked per dim-tile so the first matmuls start early
            wq_sb = wpool.tile([128, NDT, NQHc * HD], MDT, tag="wq")
            wk_sb = wpool.tile([128, NDT, HD], MDT, tag="wk")
            wv_sb = wpool.tile([128, NDT, HD], MDT, tag="wv")
            for dt in range(NDT):
                nc.sync.dma_start(out=wq_sb[:, dt, :], in_=wq_r[:, dt, :])
                nc.sync.dma_start(out=wk_sb[:, dt, :], in_=wk_r[:, dt, :])
                nc.sync.dma_start(out=wv_sb[:, dt, :], in_=wv_r[:, dt, :])

            for b in range(Bc):
                KTr = ktv.tile([128, Sc], MDT, tag="ktr")       # rotated K^T
                Vb = ktv.tile([128, NKT, 128], MDT, tag="v")     # V [tok,hd] tiles
                attnT = attnp.tile([128, NQHc, Sc], MDT, tag="attnT")
                qts = dramp.tile([NQHc, 128, Sc], MDT, tag="qts")  # QT spill

                # ================= phase 1: projections + RoPE =================
                with contextlib.ExitStack() as p1:
                    xs = p1.enter_context(tc.tile_pool(name="xs", bufs=3))
                    stg = p1.enter_context(tc.tile_pool(name="stg", bufs=2))
                    psp = p1.enter_context(
                        tc.tile_pool(name="psp", bufs=1, space="PSUM")
                    )
                    pst = p1.enter_context(
                        tc.tile_pool(name="pst", bufs=2, space="PSUM")
                    )

                    for tb in range(NB):
                        tsl = slice(tb * TB, (tb + 1) * TB)
                        psq = [psp.tile([128, TB], F32, tag=f"q{i}", name=f"psq{i}")
                               for i in range(NQHc)]
                        psk = psp.tile([128, TB], F32, tag="k")
                        psv = psp.tile([128, TB], F32, tag="vv")
                        for dt in range(NDT):
                            xt = xs.tile([128, TB], MDT, tag="xt")
                            nc.sync.dma_start(
                                out=xt,
                                in_=xT[dt * 128:(dt + 1) * 128,
                                       b * Sc + tb * TB: b * Sc + (tb + 1) * TB],
                            )
                            st_ = dt == 0
                            sp_ = dt == NDT - 1
                            for i in range(NQHc):
                                nc.tensor.matmul(
                                    psq[i], wq_sb[:, dt, i * HD:(i + 1) * HD], xt,
                                    start=st_, stop=sp_,
                                )
                            nc.tensor.matmul(psk, wk_sb[:, dt, :], xt, start=st_, stop=sp_)
                            nc.tensor.matmul(psv, wv_sb[:, dt, :], xt, start=st_, stop=sp_)

                        ct = cos_sb[:, tsl]
                        st = sin_sb[:, tsl]

                        def rope(ps_in, out_ap):
                            # out = ps_in * cos + rot_half(ps_in) * sin
                            # sinTs rows 0:64 hold -sin, rows 64:128 hold +sin.
                            tmp = stg.tile([128, TB], F32, tag="ropetmp")
                            nc.vector.tensor_mul(tmp[0:64, :], ps_in[64:128, :], st[0:64, :])
                            nc.vector.tensor_mul(tmp[64:128, :], ps_in[0:64, :], st[64:128, :])
                            tmp2 = stg.tile([128, TB], F32, tag="ropetmp2")
                            nc.vector.tensor_mul(tmp2, ps_in, ct)
                            nc.vector.tensor_add(out_ap, tmp2, tmp)

                        for i in range(NQHc):
                            qt_s = stg.tile([128, TB], MDT, tag="qt_s")
                            rope(psq[i], qt_s)
                            nc.sync.dma_start(out=qts[i, :, tsl], in_=qt_s)
                        rope(psk, KTr[:, tsl])

                        vt_s = stg.tile([128, TB], MDT, tag="vt_s")
                        nc.vector.tensor_copy(vt_s, psv)

                        for j in range(TB // 128):
                            tt = tb * (TB // 128) + j
                            tp = pst.tile([128, 128], MDT, tag="tr")
                            nc.tensor.transpose(tp, vt_s[:, j * 128:(j + 1) * 128], ident)
                            vtile = Vb[:, tt, :]
                            nc.vector.tensor_copy(vtile, tp)
                            vo_s = stg.tile([128, 128], F32, tag="vo_s")
                            nc.scalar.copy(vo_s, tp)
                            nc.sync.dma_start(
                                out=vo[b, tt * 128:(tt + 1) * 128, :], in_=vo_s
                            )
                            ktp = pst.tile([128, 128], MDT, tag="tr", name="ktp")
                            nc.tensor.transpose(
                                ktp, KTr[:, tt * 128:(tt + 1) * 128], ident
                            )
                            ko_s = stg.tile([128, 128], F32, tag="ko_s")
                            nc.scalar.copy(ko_s, ktp)
                            nc.sync.dma_start(
                                out=ko[b, tt * 128:(tt + 1) * 128, :], in_=ko_s
                            )

                # ================= phase 2: attention =================
                with contextlib.ExitStack() as p2:
                    qtp = p2.enter_context(tc.tile_pool(name="qtp", bufs=1))
                    exp_ = p2.enter_context(tc.tile_pool(name="exp", bufs=3))
                    st2 = p2.enter_context(tc.tile_pool(name="st2", bufs=2))
                    mkp = p2.enter_context(tc.tile_pool(name="mkp", bufs=1))
                    pssc = p2.enter_context(
                        tc.tile_pool(name="pssc", bufs=3, space="PSUM")
                    )
                    pspv = p2.enter_context(
                        tc.tile_pool(name="pspv", bufs=2, space="PSUM")
                    )
                    psdn = p2.enter_context(
                        tc.tile_pool(name="psdn", bufs=2, space="PSUM")
                    )
                    if cfg.causal:
                        mk_sb = mkp.tile([128, 4, TB], F32, tag="mk")
                        nc.sync.dma_start(
                            out=mk_sb, in_=maskd.rearrange("j p q -> p j q")
                        )
                    else:
                        mstr = p2.enter_context(tc.tile_pool(name="mstr", bufs=3))

                    for h in range(NQHc):
                        qth = qtp.tile([128, Sc], MDT, tag="qth")
                        nc.sync.dma_start(out=qth, in_=qts[h, :, :])
                        for qb in range(NB):
                            n_kt = 4 * (qb + 1) if cfg.causal else NKT
                            qsl = slice(qb * TB, (qb + 1) * TB)
                            pv = pspv.tile([128, TB], F32, tag="pv")
                            dn = psdn.tile([128, TB], F32, tag="dn")
                            for kt in range(n_kt):
                                sc = pssc.tile([128, TB], F32, tag="sc")
                                nc.tensor.matmul(
                                    sc, KTr[:, kt * 128:(kt + 1) * 128], qth[:, qsl],
                                    start=True, stop=True,
                                )
                                if cfg.causal:
                                    j = kt - (n_kt - 4)
                                    if j >= 0:
                                        nc.vector.tensor_add(sc, sc, mk_sb[:, j, :])
                                else:
                                    mt = mstr.tile([128, TB], F32, tag="mt")
                                    nc.sync.dma_start(out=mt, in_=maskf[qb, kt, :, :])
                                    nc.vector.tensor_add(sc, sc, mt)
                                ex = exp_.tile([128, TB], MDT, tag="ex")
                                nc.scalar.activation(
                                    ex, sc, mybir.ActivationFunctionType.Exp,
                                    bias=0.0, scale=cfg.SCALE,
                                )
                                st_ = kt == 0
                                sp_ = kt == n_kt - 1
                                nc.tensor.matmul(dn, ones, ex, start=st_, stop=sp_)
                                nc.tensor.matmul(pv, Vb[:, kt, :], ex, start=st_, stop=sp_)
                            rec = st2.tile([128, TB], F32, tag="rec")
                            nc.vector.reciprocal_approx_fast(out=rec, in_=dn)
                            nc.vector.tensor_mul(attnT[:, h, qsl], pv, rec)

                # ================= phase 3: output projection =================
                with contextlib.ExitStack() as p3:
                    wop = p3.enter_context(tc.tile_pool(name="wop", bufs=3))
                    ost = p3.enter_context(tc.tile_pool(name="ost", bufs=3))
                    pso = p3.enter_context(
                        tc.tile_pool(name="pso", bufs=4, space="PSUM")
                    )
                    for db in range(DIMc // TB):
                        dsl = slice(db * TB, (db + 1) * TB)
                        wos = wop.tile([128, NQHc, TB], MDT, tag="wos")
                        for h in range(NQHc):
                            nc.sync.dma_start(out=wos[:, h, :], in_=wo_r[:, h, dsl])
                        for tt in range(NKT):
                            po = pso.tile([128, TB], F32, tag="po")
                            for h in range(NQHc):
                                nc.tensor.matmul(
                                    po, attnT[:, h, tt * 128:(tt + 1) * 128],
                                    wos[:, h, :],
                                    start=(h == 0), stop=(h == NQHc - 1),
                                )
                            os_ = ost.tile([128, TB], F32)
                            nc.scalar.copy(os_, po)
                            nc.sync.dma_start(
                                out=outp[b, tt * 128:(tt + 1) * 128, dsl], in_=os_
                            )

    nc.compile()
    return nc


# --------------------------------------------------------------------------
# host side
# --------------------------------------------------------------------------

_CACHE = {}

DTYPE = os.environ.get("BASS_GQA_DTYPE", "bf16")


def _get_nc(causal: bool):
    key = (causal, DTYPE)
    if key not in _CACHE:
        _CACHE[key] = build_gqa_nc(Cfg(causal=causal, dtype=DTYPE))
    return _CACHE[key]


def _mdt_np():
    if DTYPE == "bf16":
        import ml_dtypes
        return ml_dtypes.bfloat16
    return np.float32


def _host_prep(x, cos, sin, mask, wq, wk, wv, wo):
    x = np.asarray(x, dtype=np.float32)
    cos = np.asarray(cos, dtype=np.float32)
    sin = np.asarray(sin, dtype=np.float32)
    mask = np.asarray(mask, dtype=np.float32)
    wq = np.asarray(wq, dtype=np.float32)
    wk = np.asarray(wk, dtype=np.float32)
    wv = np.asarray(wv, dtype=np.float32)
    wo = np.asarray(wo, dtype=np.float32)

    mdt = _mdt_np()
    xT = np.ascontiguousarray(x.reshape(B * S, DIM).T).astype(mdt)  # [DIM, B*S]
    cosT = np.ascontiguousarray(cos.T)  # [HD, S]
    sinT = np.ascontiguousarray(sin.T)
    sinTs = np.ascontiguousarray(
        np.concatenate([-sinT[: HD // 2], sinT[HD // 2:]], axis=0)
    )

    # causal structure check
    tril = np.tril(np.ones((S, S), dtype=bool))
    causal = bool(np.all(mask[tril] == 0.0) and np.all(mask[~tril] <= -1e8))

    if causal:
        TB = 512
        kk = np.arange(128)[:, None]
        qq = np.arange(TB)[None, :]
        maskd = np.stack(
            [np.where(qq >= j * 128 + kk, 0.0, -1e9).astype(np.float32)
             for j in range(4)]
        )
        mask_inputs = {"maskd": maskd}
    else:
        TB = 512
        mT = np.ascontiguousarray(mask.T)  # [k, q]
        maskf = np.ascontiguousarray(
            mT.reshape(S // 128, 128, S // TB, TB).transpose(2, 0, 1, 3)
        )
        mask_inputs = {"maskf": maskf}

    in_maps = []
    for c in range(NCORES):
        qsl = slice(c * NQH * HD, (c + 1) * NQH * HD)
        ksl = slice(c * HD, (c + 1) * HD)
        in_maps.append(
            {
                "xT": xT,
                "wq": np.ascontiguousarray(wq[:, qsl]).astype(mdt),
                "wk": np.ascontiguousarray(wk[:, ksl]).astype(mdt),
                "wv": np.ascontiguousarray(wv[:, ksl]).astype(mdt),
                "wo": np.ascontiguousarray(wo[qsl, :]).astype(mdt),
                "cosT": cosT,
                "sinTs": sinTs,
                **mask_inputs,
            }
        )
    return in_maps, causal


def run_device(x, cos, sin, mask, wq, wk, wv, wo, trace=False):
    in_maps, causal = _host_prep(x, cos, sin, mask, wq, wk, wv, wo)
    nc = _get_nc(causal)
    res = run_bass_kernel_spmd(nc, in_maps, list(range(NCORES)), trace=trace)
    return res


def kernel(x, cos, sin, mask, wq, wk, wv, wo):
    res = run_device(x, cos, sin, mask, wq, wk, wv, wo, trace=False)
    outs = res.results
    out = np.zeros((B, S, DIM), dtype=np.float64)
    for c in range(NCORES):
        out += outs[c]["outp"].astype(np.float64)
    out = out.astype(np.float32)
    k = np.stack([outs[c]["ko"] for c in range(NCORES)], axis=1)  # [B,NKV,S,HD]
    v = np.stack([outs[c]["vo"] for c in range(NCORES)], axis=1)
    return out, k, v
